# revision 23
# baseline (speedup 1.0000x reference)
"""Trainium2 Bass kernel for nn_CfdGinoMeshToGridOld (gather + MLP + segment
mean, sharded by grid-segment across 8 NeuronCores; no collectives needed
since grid_idx is sorted and segments partition cleanly by value)."""

import ml_dtypes
import numpy as np
import concourse.bass as bass
import concourse.tile as tile
from concourse import bacc, mybir
from concourse import bass_utils
from contextlib import ExitStack


N_CORES = 8
G = 32768
HID = 384
BIN_E = 128          # edge slots per bin
BIN_S = 64           # segment slots per bin
TILE = 512           # slots per e-tile
BIN_ROUND = 12       # nbins must divide into e-tiles (4) and seg blocks (6)


def pack(mesh_to_grid_edges, x, mesh_pos, grid_pos):
    """Partition edges by (sorted) grid id into 8 cores, FFD bin-pack
    segments, and build all per-core device input arrays + scatter maps.
    Returns (per_core, nbins)."""
    gidx = np.asarray(mesh_to_grid_edges[:, 0], dtype=np.int64)
    midx = np.asarray(mesh_to_grid_edges[:, 1], dtype=np.int64)
    order = np.argsort(gidx, kind="stable")
    gidx, midx = gidx[order], midx[order]
    E = gidx.shape[0]

    counts = np.bincount(gidx, minlength=G)
    nz = np.flatnonzero(counts)
    sizes = counts[nz]
    starts = np.concatenate([[0], np.cumsum(sizes)[:-1]])

    core_of_seg = np.minimum(starts * N_CORES // E, N_CORES - 1)

    packed = []
    for c in range(N_CORES):
        segs = np.flatnonzero(core_of_seg == c)
        # split oversize segments into <=BIN_E chunks
        items = []  # (gid, edge_start, size, is_extra)
        for s in segs:
            g, size, e0 = int(nz[s]), int(sizes[s]), int(starts[s])
            off = 0
            while size - off > BIN_E:
                items.append((g, e0 + off, BIN_E, off > 0))
                off += BIN_E
            items.append((g, e0 + off, size - off, off > 0))
        # FFD
        items.sort(key=lambda it: -it[2])
        bins = []  # [edges_used, [items]]
        for it in items:
            placed = False
            for bn in bins:
                if bn[0] + it[2] <= BIN_E and len(bn[1]) < BIN_S:
                    bn[0] += it[2]
                    bn[1].append(it)
                    placed = True
                    break
            if not placed:
                bins.append([it[2], [it]])
        packed.append(bins)

    nbins = max(len(b) for b in packed)
    nbins = ((nbins + BIN_ROUND - 1) // BIN_ROUND) * BIN_ROUND
    S = nbins * BIN_E
    NSEG = nbins * BIN_S
    T = S // TILE

    per_core = []
    for c in range(N_CORES):
        bins = packed[c]
        slot_mesh = np.zeros(S, dtype=np.int64)
        slot_gid = np.zeros(S, dtype=np.int64)
        slot_valid = np.zeros(S, dtype=bool)
        sel = np.zeros((nbins, BIN_E, 2 * BIN_S), dtype=np.float32)
        segrow_gid = np.full(NSEG, -1, dtype=np.int64)
        segrow_extra = np.zeros(NSEG, dtype=bool)
        for b, (_, its) in enumerate(bins):
            be = 0
            for bs, (g, e0, size, extra) in enumerate(its):
                rows = slice(b * BIN_E + be, b * BIN_E + be + size)
                slot_mesh[rows] = midx[e0 : e0 + size]
                slot_gid[rows] = g
                slot_valid[rows] = True
                sel[b, be : be + size, (b % 2) * BIN_S + bs] = 1.0 / counts[g]
                segrow_gid[b * BIN_S + bs] = g
                segrow_extra[b * BIN_S + bs] = extra
                be += size
        pc = dict(
            slot_mesh=slot_mesh, slot_gid=slot_gid, slot_valid=slot_valid,
            sel=sel, segrow_gid=segrow_gid, segrow_extra=segrow_extra,
            used_bins=len(bins), nbins=nbins, nseg=NSEG, s_slots=S, t_tiles=T,
        )
        per_core.append(pc)

    fcoord, _, _, fmesh = _pe_feature_table()
    for pc in per_core:
        sm, sg, sv = pc["slot_mesh"], pc["slot_gid"], pc["slot_valid"]
        xT = (x[sm] * sv[:, None]).T.astype(np.float32)
        mp = (mesh_pos[sm] * sv[:, None]).astype(np.float32)   # [S, 3]
        gp = (grid_pos[sg] * sv[:, None]).astype(np.float32)   # [S, 3]
        # rep3[f, slot] = coord value of pe-feature f (384 features)
        src_coords = np.where(fmesh[:, None], mp.T[fcoord], gp.T[fcoord])  # [384, S]
        pc["xT_t"] = np.ascontiguousarray(
            xT.reshape(16, T, TILE).transpose(1, 0, 2)).astype(ml_dtypes.bfloat16)
        pc["rep3_t"] = np.ascontiguousarray(
            src_coords.reshape(3, 128, T, TILE).transpose(2, 1, 0, 3))  # [T,128,3,512]
        pc["sel"] = pc["sel"].astype(ml_dtypes.bfloat16)
    return per_core, nbins


def _pe_feature_table():
    """384 pe-features: f 0..95 mesh-sin, 96..191 mesh-cos, 192..287 grid-sin,
    288..383 grid-cos; within a 96-block: coord c = i//32, freq = i%32.
    Chunk layout: feature f lives at partition f%128, chunk f//128."""
    f = np.arange(384)
    blk = f // 96            # 0 msin 1 mcos 2 gsin 3 gcos
    i = f % 96
    fcoord = i // 32
    ffreq = i % 32
    fphase = np.where(blk % 2 == 1, np.pi / 2, 0.0)
    fmesh = blk < 2
    return fcoord, ffreq, fphase, fmesh


def make_weights(inp):
    """Host-side weight re-arrangements (pure reshapes/permutes).

    h3 has no activation before the message MLP, so w_in3/b_in3 fold into
    the h-half of w_m1: h3 @ w_m1[:384] == h2 @ (w_in3 @ w_m1[:384]) +
    b_in3 @ w_m1[:384]. Saves the whole h3 matmul stage on device."""
    w = {}
    w_m1 = np.asarray(inp["w_m1"], dtype=np.float32)
    w_in3 = np.asarray(inp["w_in3"], np.float32)
    b_in3 = np.asarray(inp["b_in3"], np.float32)
    w["w_in1"] = np.asarray(inp["w_in1"], np.float32).astype(ml_dtypes.bfloat16)
    w["w_in2"] = np.asarray(inp["w_in2"], np.float32).reshape(3, 128, 384).transpose(1, 0, 2).astype(ml_dtypes.bfloat16)
    w_m1h_fused = w_in3 @ w_m1[:384]                                      # [384,768]
    w["w_m1h"] = w_m1h_fused.reshape(3, 128, 768).transpose(1, 0, 2).astype(ml_dtypes.bfloat16)
    fcoord, ffreq, fphase, fmesh = _pe_feature_table()
    # original w_m1 row for pe-feature f: base 384 (mesh) / 576 (grid),
    # offset coord*64 + freq (+32 for cos)
    cos_off = np.where(fphase > 0, 32, 0)
    rows = np.where(fmesh, 384, 576) + fcoord * 64 + ffreq + cos_off
    w_pe3 = w_m1[rows]                                   # [384, 768]
    w["w_pe3"] = np.ascontiguousarray(
        w_pe3.reshape(3, 128, 768).transpose(1, 0, 2)).astype(ml_dtypes.bfloat16)
    eff = 64
    omega_f = (1.0 / 10000.0 ** (np.arange(0, eff, 2) / eff)).astype(np.float32)
    w["omega3"] = np.ascontiguousarray(
        omega_f[ffreq].reshape(3, 128).T).astype(np.float32)      # [128, 3]
    w["phase3"] = np.ascontiguousarray(
        fphase.reshape(3, 128).T).astype(np.float32)              # [128, 3]
    w["w_m2"] = np.asarray(inp["w_m2"], np.float32).reshape(6, 128, 768).transpose(1, 0, 2).astype(ml_dtypes.bfloat16)
    w["w_m3"] = np.asarray(inp["w_m3"], np.float32).reshape(6, 128, 384).transpose(1, 0, 2).astype(ml_dtypes.bfloat16)
    w["b_in1"] = np.asarray(inp["b_in1"], np.float32).reshape(3, 128).T.copy()  # [128,3]
    w["b_in2"] = np.asarray(inp["b_in2"], np.float32).reshape(3, 128).T.copy()
    b_m1_fused = b_in3 @ w_m1[:384] + np.asarray(inp["b_m1"], np.float32)
    w["b_m1"] = b_m1_fused.reshape(6, 128).T.copy()                             # [128,6]
    w["b_m2_rep"] = np.tile(np.asarray(inp["b_m2"], np.float32), (128, 1))      # [128,768]
    w["b_m3"] = np.asarray(inp["b_m3"], np.float32).reshape(3, 128).T.copy()    # [128,3]
    w["ident"] = np.eye(128, dtype=ml_dtypes.bfloat16)
    return w


def assemble(per_core, outs_rows, b_m3_full):
    """Scatter per-core compact rows into the [G, HID] output."""
    full = np.zeros((G, HID), dtype=np.float32)
    for pc, rows in zip(per_core, outs_rows):
        gids = pc["segrow_gid"]
        extra = pc["segrow_extra"]
        valid = gids >= 0
        r = rows.copy()
        r[extra & valid] -= b_m3_full[None, :]
        np.add.at(full, gids[valid], r[valid])
    return full.reshape(1, G, HID)



F32 = mybir.dt.float32
F32R = mybir.dt.float32r
BF16 = mybir.dt.bfloat16
I32 = mybir.dt.int32
GELU = mybir.ActivationFunctionType.Gelu
IDENT = mybir.ActivationFunctionType.Identity
SIN = mybir.ActivationFunctionType.Sin

TWO_PI = 2.0 * np.pi
INV_2PI = float(1.0 / TWO_PI)
CW1 = 6.28125
CW2 = float(np.float32(TWO_PI - 6.28125))
CW3 = float(TWO_PI - 6.28125 - np.float32(TWO_PI - 6.28125))

BIN_E = 128
BIN_S = 64
TILE_SLOTS = 512
BINS_PER_TILE = TILE_SLOTS // BIN_E          # 4
SEG_BLOCK = 384
BINS_PER_SEGBLOCK = SEG_BLOCK // BIN_S        # 6


def build_nc(nbins, debug=False):
    assert nbins % BINS_PER_SEGBLOCK == 0
    t_tiles = nbins * BIN_E // TILE_SLOTS
    nseg = nbins * BIN_S

    nc = bacc.Bacc("TRN2", target_bir_lowering=False, debug=debug)

    # ---- DRAM I/O ----
    d_xT = nc.dram_tensor("xT_t", [t_tiles, 16, TILE_SLOTS], BF16, kind="ExternalInput")
    d_rep3 = nc.dram_tensor("rep3_t", [t_tiles, 128, 3, TILE_SLOTS], F32, kind="ExternalInput")
    d_sel = nc.dram_tensor("sel_t", [nbins, BIN_E, 2 * BIN_S], BF16, kind="ExternalInput")
    d_w_in1 = nc.dram_tensor("w_in1", [16, 384], BF16, kind="ExternalInput")
    d_w_in2 = nc.dram_tensor("w_in2", [128, 3, 384], BF16, kind="ExternalInput")
    d_w_m1h = nc.dram_tensor("w_m1h", [128, 3, 768], BF16, kind="ExternalInput")
    d_w_pe3 = nc.dram_tensor("w_pe3", [128, 3, 768], BF16, kind="ExternalInput")
    d_w_m2 = nc.dram_tensor("w_m2", [128, 6, 768], BF16, kind="ExternalInput")
    d_w_m3 = nc.dram_tensor("w_m3", [128, 6, 384], BF16, kind="ExternalInput")
    d_b_in1 = nc.dram_tensor("b_in1", [128, 3], F32, kind="ExternalInput")
    d_b_in2 = nc.dram_tensor("b_in2", [128, 3], F32, kind="ExternalInput")
    d_b_m1 = nc.dram_tensor("b_m1", [128, 6], F32, kind="ExternalInput")
    d_b_m2r = nc.dram_tensor("b_m2_rep", [128, 768], F32, kind="ExternalInput")
    d_b_m3 = nc.dram_tensor("b_m3", [128, 3], F32, kind="ExternalInput")
    d_omega3 = nc.dram_tensor("omega3", [128, 3], F32, kind="ExternalInput")
    d_phase3 = nc.dram_tensor("phase3", [128, 3], F32, kind="ExternalInput")
    d_ident = nc.dram_tensor("ident", [128, 128], BF16, kind="ExternalInput")
    d_out = nc.dram_tensor("outT", [3, 128, nseg], F32, kind="ExternalOutput")

    with tile.TileContext(nc) as tc:
        with ExitStack() as ctx:
            ent = ctx.enter_context
            wp = ent(tc.tile_pool(name="wp", bufs=1))
            xin_p = ent(tc.tile_pool(name="xin", bufs=4))
            rep_p = ent(tc.tile_pool(name="rep", bufs=3))
            trig_p = ent(tc.tile_pool(name="trig", bufs=2))
            sc_p = ent(tc.tile_pool(name="sc", bufs=4))
            h1_p = ent(tc.tile_pool(name="h1p", bufs=4))
            h2_p = ent(tc.tile_pool(name="h2p", bufs=4))
            tT_p = ent(tc.tile_pool(name="tTp", bufs=8))
            sel_p = ent(tc.tile_pool(name="selp", bufs=8))
            m2a_p = ent(tc.tile_pool(name="m2ap", bufs=2))
            m2g_p = ent(tc.tile_pool(name="m2gp", bufs=5))
            sm_p = ent(tc.tile_pool(name="smp", bufs=3))
            smT_p = ent(tc.tile_pool(name="smTp", bufs=12))
            out_p = ent(tc.tile_pool(name="outp", bufs=4))
            psA = ent(tc.tile_pool(name="psA", bufs=4, space=bass.MemorySpace.PSUM))
            psE = ent(tc.tile_pool(name="psE", bufs=4, space=bass.MemorySpace.PSUM))

            def wload(dram, shape, dt):
                t = wp.tile(shape, dt, tag=dram.name, name=dram.name + "_sb")
                nc.sync.dma_start(t[:], dram[:])
                return t

            w_in1 = wload(d_w_in1, [16, 384], BF16)
            b_in1 = wload(d_b_in1, [128, 3], F32)
            b_in2 = wload(d_b_in2, [128, 3], F32)
            b_m1 = wload(d_b_m1, [128, 6], F32)
            b_m3 = wload(d_b_m3, [128, 3], F32)
            omega3 = wload(d_omega3, [128, 3], F32)
            phase3 = wload(d_phase3, [128, 3], F32)
            ident = wload(d_ident, [128, 128], BF16)
            w_in2 = wload(d_w_in2, [128, 3, 384], BF16)
            w_m1h = wload(d_w_m1h, [128, 3, 768], BF16)
            w_pe3 = wload(d_w_pe3, [128, 3, 768], BF16)
            b_m2r = wload(d_b_m2r, [128, 768], F32)
            w_m2 = wload(d_w_m2, [128, 6, 768], BF16)
            w_m3 = wload(d_w_m3, [128, 6, 384], BF16)

            pair_ps = [None, None]
            smT_tiles = {}
            pending = []

            def emit_seg(b, m2g):
                selt = sel_p.tile([BIN_E, 2 * BIN_S], BF16, tag="sel", name="sel")
                nc.sync.dma_start(selt[:], d_sel[b])
                half = b % 2
                if half == 0:
                    pair_ps[0] = psE.tile([128, 384], F32, tag="psE", name="psE")
                    pair_ps[1] = psE.tile([128, 384], F32, tag="psE", name="psE")
                pSa, pSb = pair_ps
                # both bins of the pair accumulate into one PSUM pair
                nc.tensor.matmul(pSa[:], selt[:], m2g[:, 0:384],
                                 start=(half == 0), stop=(half == 1))
                nc.tensor.matmul(pSb[:], selt[:], m2g[:, 384:768],
                                 start=(half == 0), stop=(half == 1))
                if half == 0:
                    return
                sm = sm_p.tile([128, 768], BF16, tag="sm", name="sm")
                nc.vector.tensor_copy(sm[:, 0:384], pSa[:])
                nc.vector.tensor_copy(sm[:, 384:768], pSb[:])

                # 128 seg rows complete -> 6 transposes into smeanT
                grp = b // 2
                q = grp % 3
                for kc in range(6):
                    if q == 0:
                        smT_tiles[kc] = smT_p.tile(
                            [128, SEG_BLOCK], BF16, tag="smT", name="smT"
                        )
                    ptr = psA.tile([128, 128], BF16, tag="psA", name="ptr")
                    nc.tensor.transpose(ptr[:], sm[:, bass.ts(kc, 128)], ident[:])
                    nc.vector.tensor_copy(smT_tiles[kc][:, bass.ts(q, 128)], ptr[:])
                if q == 2:
                    sb = grp // 3
                    for j in range(3):
                        ps = psA.tile([128, SEG_BLOCK], F32, tag="psA", name="psA")
                        for kc in range(6):
                            nc.tensor.matmul(
                                ps[:], w_m3[:, kc, bass.ts(j, 128)],
                                smT_tiles[kc][:],
                                start=(kc == 0), stop=(kc == 5),
                            )
                        ot = out_p.tile([128, SEG_BLOCK], F32, tag="out", name="out")
                        nc.vector.tensor_scalar_add(ot[:], ps[:], b_m3[:, j : j + 1])
                        nc.sync.dma_start(d_out[j, :, bass.ts(sb, SEG_BLOCK)], ot[:])


            def trig_reduce(rep_t, c, dst, off):
                """rep3 chunk c -> range-reduced args into dst[:, c, off:]."""
                arg = trig_p.tile([128, TILE_SLOTS], F32, tag="arg", name="arg")
                nc.vector.tensor_scalar(arg[:], rep_t[:, c, :],
                                        omega3[:, c : c + 1],
                                        phase3[:, c : c + 1],
                                        op0=mybir.AluOpType.mult,
                                        op1=mybir.AluOpType.add)
                ki = trig_p.tile([128, TILE_SLOTS], I32, tag="ki", name="ki")
                nc.vector.tensor_scalar_mul(ki[:], arg[:], INV_2PI)
                kf = trig_p.tile([128, TILE_SLOTS], F32, tag="kf", name="kf")
                nc.vector.tensor_copy(kf[:], ki[:])
                nc.vector.cody_waite_cascade(dst[:, c, off : off + TILE_SLOTS],
                                             arg[:], kf[:], CW1, CW2, CW3)

            def pair_front(tis):
                # ---- input DMAs + trig range-reduction (DVE) + ONE SIN for
                # the whole pair (one activation-table swap pair per 2 tiles)
                n = len(tis)
                xTs, reps = [], []
                for ti in tis:
                    xT = xin_p.tile([16, TILE_SLOTS], BF16, tag="xin", name="xin")
                    nc.sync.dma_start(xT[:], d_xT[ti])
                    xTs.append(xT)
                    rep_t = rep_p.tile([128, 3, TILE_SLOTS], F32, tag="rep", name="rep")
                    nc.sync.dma_start(rep_t[:], d_rep3[ti])
                    reps.append(rep_t)
                rr = trig_p.tile([128, 3, n * TILE_SLOTS], F32,
                                 tag=f"rr{n}", name="rr")
                for c in range(3):
                    for k, rep_t in enumerate(reps):
                        trig_reduce(rep_t, c, rr, k * TILE_SLOTS)
                sc = sc_p.tile([128, 3, n * TILE_SLOTS], BF16,
                               tag=f"sc{n}", name="sc")
                nc.scalar.activation(sc[:], rr[:], SIN)
                return xTs, sc

            def tile_body(ti, xT, sc, koff):
                # ---- node MLP (feature-major) ----
                h1 = []
                for j in range(3):
                    ps = psA.tile([128, TILE_SLOTS], F32, tag="psA", name="psA")
                    nc.tensor.matmul(ps[:], w_in1[:, bass.ts(j, 128)], xT[:])
                    t = h1_p.tile([128, TILE_SLOTS], BF16, tag="h1", name="h1")
                    nc.scalar.activation(t[:], ps[:], GELU, bias=b_in1[:, j : j + 1])
                    h1.append(t)
                h2 = []
                for j in range(3):
                    ps = psA.tile([128, TILE_SLOTS], F32, tag="psA", name="psA")
                    for kc in range(3):
                        nc.tensor.matmul(
                            ps[:], w_in2[:, kc, bass.ts(j, 128)], h1[kc][:],
                            start=(kc == 0), stop=(kc == 2),
                        )
                    t = h2_p.tile([128, TILE_SLOTS], BF16, tag="h2", name="h2")
                    nc.scalar.activation(t[:], ps[:], GELU, bias=b_in2[:, j : j + 1])
                    h2.append(t)
                # h3 is fused into w_m1h host-side (no activation in between)
                tT = []
                for f in range(6):
                    ps = psA.tile([128, TILE_SLOTS], F32, tag="psA", name="psA")
                    for kc in range(3):
                        nc.tensor.matmul(
                            ps[:], w_m1h[:, kc, bass.ts(f, 128)], h2[kc][:],
                            start=(kc == 0), stop=False,
                        )
                    for c in range(3):
                        nc.tensor.matmul(
                            ps[:], w_pe3[:, c, bass.ts(f, 128)],
                            sc[:, c, koff : koff + TILE_SLOTS],
                            start=False, stop=(c == 2),
                        )
                    t = tT_p.tile([128, TILE_SLOTS], BF16, tag="tT", name="tT")
                    nc.scalar.activation(t[:], ps[:], GELU, bias=b_m1[:, f : f + 1])
                    tT.append(t)

                # ---- edge stage: 4 bins (m2 part) ----
                for bi in range(BINS_PER_TILE):
                    b = ti * BINS_PER_TILE + bi
                    esl = bass.ts(bi, BIN_E)
                    pA = psE.tile([128, 384], F32, tag="psE", name="psE")
                    pB = psE.tile([128, 384], F32, tag="psE", name="psE")
                    for kc in range(6):
                        nc.tensor.matmul(
                            pA[:], tT[kc][:, esl], w_m2[:, kc, 0:384],
                            start=(kc == 0), stop=(kc == 5),
                        )
                    for kc in range(6):
                        nc.tensor.matmul(
                            pB[:], tT[kc][:, esl], w_m2[:, kc, 384:768],
                            start=(kc == 0), stop=(kc == 5),
                        )
                    m2a = m2a_p.tile([128, 768], F32, tag="m2a", name="m2a")
                    nc.vector.tensor_add(m2a[:, 0:384], pA[:], b_m2r[:, 0:384])
                    nc.vector.tensor_add(m2a[:, 384:768], pB[:], b_m2r[:, 384:768])
                    m2g = m2g_p.tile([128, 768], BF16, tag="m2g", name="m2g")
                    nc.scalar.activation(m2g[:], m2a[:], GELU)
                    pending.append((b, m2g))
                    if len(pending) > 3:
                        emit_seg(*pending.pop(0))

            # pair tiles so the two SINs are adjacent on the scalar queue
            # (one SIN<->GELU activation-table swap pair per TWO tiles)
            for tp in range(0, t_tiles, 2):
                tis = list(range(tp, min(tp + 2, t_tiles)))
                xTs, sc = pair_front(tis)
                for k, ti in enumerate(tis):
                    tile_body(ti, xTs[k], sc, k * TILE_SLOTS)
            # drain the pipeline
            while pending:
                emit_seg(*pending.pop(0))
    nc.compile()
    return nc


_NC_CACHE = {}


def _get_nc(nbins):
    if nbins not in _NC_CACHE:
        _NC_CACHE[nbins] = build_nc(nbins)
    return _NC_CACHE[nbins]


def kernel(**inputs):
    x = np.asarray(inputs["x"], np.float32)
    mesh_pos = np.asarray(inputs["mesh_pos"], np.float32)
    grid_pos = np.asarray(inputs["grid_pos"], np.float32)
    edges = np.asarray(inputs["mesh_to_grid_edges"])

    per_core, nbins = pack(edges, x, mesh_pos, grid_pos)
    w = make_weights(inputs)
    nc = _get_nc(nbins)

    common = dict(
        w_in1=w["w_in1"], w_in2=np.ascontiguousarray(w["w_in2"]),
        w_m1h=np.ascontiguousarray(w["w_m1h"]),
        w_pe3=np.ascontiguousarray(w["w_pe3"]),
        w_m2=np.ascontiguousarray(w["w_m2"]),
        w_m3=np.ascontiguousarray(w["w_m3"]),
        b_in1=w["b_in1"], b_in2=w["b_in2"], b_m1=w["b_m1"],
        b_m2_rep=w["b_m2_rep"], b_m3=w["b_m3"],
        omega3=w["omega3"], phase3=w["phase3"], ident=w["ident"],
    )
    in_maps = [dict(common, xT_t=pc["xT_t"], rep3_t=pc["rep3_t"],
                    sel_t=pc["sel"]) for pc in per_core]

    res = bass_utils.run_bass_kernel_spmd(nc, in_maps, core_ids=list(range(N_CORES)))
    outs_rows = [r["outT"].transpose(2, 0, 1).reshape(per_core[0]["nseg"], HID)
                 for r in res.results]
    return assemble(per_core, outs_rows, np.asarray(inputs["b_m3"], np.float32))



# revision 24
# speedup vs baseline: 1.0294x; 1.0294x over previous
"""Trainium2 Bass kernel for nn_CfdGinoMeshToGridOld (gather + MLP + segment
mean, sharded by grid-segment across 8 NeuronCores; no collectives needed
since grid_idx is sorted and segments partition cleanly by value)."""

import ml_dtypes
import numpy as np
import concourse.bass as bass
import concourse.tile as tile
from concourse import bacc, mybir
from concourse import bass_utils
from contextlib import ExitStack


N_CORES = 8
G = 32768
HID = 384
BIN_E = 128          # edge slots per bin
BIN_S = 64           # segment slots per bin
TILE = 512           # slots per e-tile
BIN_ROUND = 12       # nbins must divide into e-tiles (4) and seg blocks (6)


def pack(mesh_to_grid_edges, x, mesh_pos, grid_pos):
    """Partition edges by (sorted) grid id into 8 cores, FFD bin-pack
    segments, and build all per-core device input arrays + scatter maps.
    Returns (per_core, nbins)."""
    gidx = np.asarray(mesh_to_grid_edges[:, 0], dtype=np.int64)
    midx = np.asarray(mesh_to_grid_edges[:, 1], dtype=np.int64)
    order = np.argsort(gidx, kind="stable")
    gidx, midx = gidx[order], midx[order]
    E = gidx.shape[0]

    counts = np.bincount(gidx, minlength=G)
    nz = np.flatnonzero(counts)
    sizes = counts[nz]
    starts = np.concatenate([[0], np.cumsum(sizes)[:-1]])

    core_of_seg = np.minimum(starts * N_CORES // E, N_CORES - 1)

    packed = []
    for c in range(N_CORES):
        segs = np.flatnonzero(core_of_seg == c)
        # split oversize segments into <=BIN_E chunks
        items = []  # (gid, edge_start, size, is_extra)
        for s in segs:
            g, size, e0 = int(nz[s]), int(sizes[s]), int(starts[s])
            off = 0
            while size - off > BIN_E:
                items.append((g, e0 + off, BIN_E, off > 0))
                off += BIN_E
            items.append((g, e0 + off, size - off, off > 0))
        # FFD
        items.sort(key=lambda it: -it[2])
        bins = []  # [edges_used, [items]]
        for it in items:
            placed = False
            for bn in bins:
                if bn[0] + it[2] <= BIN_E and len(bn[1]) < BIN_S:
                    bn[0] += it[2]
                    bn[1].append(it)
                    placed = True
                    break
            if not placed:
                bins.append([it[2], [it]])
        packed.append(bins)

    nbins = max(len(b) for b in packed)
    nbins = ((nbins + BIN_ROUND - 1) // BIN_ROUND) * BIN_ROUND
    S = nbins * BIN_E
    NSEG = nbins * BIN_S
    T = S // TILE

    per_core = []
    for c in range(N_CORES):
        bins = packed[c]
        slot_mesh = np.zeros(S, dtype=np.int64)
        slot_gid = np.zeros(S, dtype=np.int64)
        slot_valid = np.zeros(S, dtype=bool)
        sel = np.zeros((nbins, BIN_E, 2 * BIN_S), dtype=np.float32)
        segrow_gid = np.full(NSEG, -1, dtype=np.int64)
        segrow_extra = np.zeros(NSEG, dtype=bool)
        for b, (_, its) in enumerate(bins):
            be = 0
            for bs, (g, e0, size, extra) in enumerate(its):
                rows = slice(b * BIN_E + be, b * BIN_E + be + size)
                slot_mesh[rows] = midx[e0 : e0 + size]
                slot_gid[rows] = g
                slot_valid[rows] = True
                sel[b, be : be + size, (b % 2) * BIN_S + bs] = 1.0 / counts[g]
                segrow_gid[b * BIN_S + bs] = g
                segrow_extra[b * BIN_S + bs] = extra
                be += size
        pc = dict(
            slot_mesh=slot_mesh, slot_gid=slot_gid, slot_valid=slot_valid,
            sel=sel, segrow_gid=segrow_gid, segrow_extra=segrow_extra,
            used_bins=len(bins), nbins=nbins, nseg=NSEG, s_slots=S, t_tiles=T,
        )
        per_core.append(pc)

    fcoord, _, _, fmesh = _pe_feature_table()
    for pc in per_core:
        sm, sg, sv = pc["slot_mesh"], pc["slot_gid"], pc["slot_valid"]
        xT = (x[sm] * sv[:, None]).T.astype(np.float32)
        mp = (mesh_pos[sm] * sv[:, None]).astype(np.float32)   # [S, 3]
        gp = (grid_pos[sg] * sv[:, None]).astype(np.float32)   # [S, 3]
        # rep3[f, slot] = coord value of pe-feature f (384 features)
        src_coords = np.where(fmesh[:, None], mp.T[fcoord], gp.T[fcoord])  # [384, S]
        pc["xT_t"] = np.ascontiguousarray(
            xT.reshape(16, T, TILE).transpose(1, 0, 2)).astype(ml_dtypes.bfloat16)
        pc["rep3_t"] = np.ascontiguousarray(
            src_coords.reshape(3, 128, T, TILE).transpose(2, 1, 0, 3))  # [T,128,3,512]
        pc["sel"] = pc["sel"].astype(ml_dtypes.bfloat16)
    return per_core, nbins


def _pe_feature_table():
    """384 pe-features: f 0..95 mesh-sin, 96..191 mesh-cos, 192..287 grid-sin,
    288..383 grid-cos; within a 96-block: coord c = i//32, freq = i%32.
    Chunk layout: feature f lives at partition f%128, chunk f//128."""
    f = np.arange(384)
    blk = f // 96            # 0 msin 1 mcos 2 gsin 3 gcos
    i = f % 96
    fcoord = i // 32
    ffreq = i % 32
    fphase = np.where(blk % 2 == 1, np.pi / 2, 0.0)
    fmesh = blk < 2
    return fcoord, ffreq, fphase, fmesh


def make_weights(inp):
    """Host-side weight re-arrangements (pure reshapes/permutes).

    h3 has no activation before the message MLP, so w_in3/b_in3 fold into
    the h-half of w_m1: h3 @ w_m1[:384] == h2 @ (w_in3 @ w_m1[:384]) +
    b_in3 @ w_m1[:384]. Saves the whole h3 matmul stage on device."""
    w = {}
    w_m1 = np.asarray(inp["w_m1"], dtype=np.float32)
    w_in3 = np.asarray(inp["w_in3"], np.float32)
    b_in3 = np.asarray(inp["b_in3"], np.float32)
    w["w_in1"] = np.asarray(inp["w_in1"], np.float32).astype(ml_dtypes.bfloat16)
    w["w_in2"] = np.asarray(inp["w_in2"], np.float32).reshape(3, 128, 384).transpose(1, 0, 2).astype(ml_dtypes.bfloat16)
    w_m1h_fused = w_in3 @ w_m1[:384]                                      # [384,768]
    w["w_m1h"] = w_m1h_fused.reshape(3, 128, 768).transpose(1, 0, 2).astype(ml_dtypes.bfloat16)
    fcoord, ffreq, fphase, fmesh = _pe_feature_table()
    # original w_m1 row for pe-feature f: base 384 (mesh) / 576 (grid),
    # offset coord*64 + freq (+32 for cos)
    cos_off = np.where(fphase > 0, 32, 0)
    rows = np.where(fmesh, 384, 576) + fcoord * 64 + ffreq + cos_off
    w_pe3 = w_m1[rows]                                   # [384, 768]
    w["w_pe3"] = np.ascontiguousarray(
        w_pe3.reshape(3, 128, 768).transpose(1, 0, 2)).astype(ml_dtypes.bfloat16)
    eff = 64
    omega_f = (1.0 / 10000.0 ** (np.arange(0, eff, 2) / eff)).astype(np.float32)
    w["omega3"] = np.ascontiguousarray(
        omega_f[ffreq].reshape(3, 128).T).astype(np.float32)      # [128, 3]
    w["phase3"] = np.ascontiguousarray(
        fphase.reshape(3, 128).T).astype(np.float32)              # [128, 3]
    w["w_m2"] = np.asarray(inp["w_m2"], np.float32).reshape(6, 128, 768).transpose(1, 0, 2).astype(ml_dtypes.bfloat16)
    w["w_m3"] = np.asarray(inp["w_m3"], np.float32).reshape(6, 128, 384).transpose(1, 0, 2).astype(ml_dtypes.bfloat16)
    w["b_in1"] = np.asarray(inp["b_in1"], np.float32).reshape(3, 128).T.copy()  # [128,3]
    w["b_in2"] = np.asarray(inp["b_in2"], np.float32).reshape(3, 128).T.copy()
    b_m1_fused = b_in3 @ w_m1[:384] + np.asarray(inp["b_m1"], np.float32)
    w["b_m1"] = b_m1_fused.reshape(6, 128).T.copy()                             # [128,6]
    w["b_m2_rep"] = np.tile(np.asarray(inp["b_m2"], np.float32), (128, 1))      # [128,768]
    w["b_m3"] = np.asarray(inp["b_m3"], np.float32).reshape(3, 128).T.copy()    # [128,3]
    w["ident"] = np.eye(128, dtype=ml_dtypes.bfloat16)
    return w


def assemble(per_core, outs_rows, b_m3_full):
    """Scatter per-core compact rows into the [G, HID] output."""
    full = np.zeros((G, HID), dtype=np.float32)
    for pc, rows in zip(per_core, outs_rows):
        gids = pc["segrow_gid"]
        extra = pc["segrow_extra"]
        valid = gids >= 0
        r = rows.copy()
        r[extra & valid] -= b_m3_full[None, :]
        np.add.at(full, gids[valid], r[valid])
    return full.reshape(1, G, HID)



F32 = mybir.dt.float32
F32R = mybir.dt.float32r
BF16 = mybir.dt.bfloat16
I32 = mybir.dt.int32
GELU = mybir.ActivationFunctionType.Gelu
IDENT = mybir.ActivationFunctionType.Identity
SIN = mybir.ActivationFunctionType.Sin

TWO_PI = 2.0 * np.pi
INV_2PI = float(1.0 / TWO_PI)
CW1 = 6.28125
CW2 = float(np.float32(TWO_PI - 6.28125))
CW3 = float(TWO_PI - 6.28125 - np.float32(TWO_PI - 6.28125))

BIN_E = 128
BIN_S = 64
TILE_SLOTS = 512
BINS_PER_TILE = TILE_SLOTS // BIN_E          # 4
SEG_BLOCK = 384
BINS_PER_SEGBLOCK = SEG_BLOCK // BIN_S        # 6


def build_nc(nbins, debug=False):
    assert nbins % BINS_PER_SEGBLOCK == 0
    t_tiles = nbins * BIN_E // TILE_SLOTS
    nseg = nbins * BIN_S

    nc = bacc.Bacc("TRN2", target_bir_lowering=False, debug=debug)

    # ---- DRAM I/O ----
    d_xT = nc.dram_tensor("xT_t", [t_tiles, 16, TILE_SLOTS], BF16, kind="ExternalInput")
    d_rep3 = nc.dram_tensor("rep3_t", [t_tiles, 128, 3, TILE_SLOTS], F32, kind="ExternalInput")
    d_sel = nc.dram_tensor("sel_t", [nbins, BIN_E, 2 * BIN_S], BF16, kind="ExternalInput")
    d_w_in1 = nc.dram_tensor("w_in1", [16, 384], BF16, kind="ExternalInput")
    d_w_in2 = nc.dram_tensor("w_in2", [128, 3, 384], BF16, kind="ExternalInput")
    d_w_m1h = nc.dram_tensor("w_m1h", [128, 3, 768], BF16, kind="ExternalInput")
    d_w_pe3 = nc.dram_tensor("w_pe3", [128, 3, 768], BF16, kind="ExternalInput")
    d_w_m2 = nc.dram_tensor("w_m2", [128, 6, 768], BF16, kind="ExternalInput")
    d_w_m3 = nc.dram_tensor("w_m3", [128, 6, 384], BF16, kind="ExternalInput")
    d_b_in1 = nc.dram_tensor("b_in1", [128, 3], F32, kind="ExternalInput")
    d_b_in2 = nc.dram_tensor("b_in2", [128, 3], F32, kind="ExternalInput")
    d_b_m1 = nc.dram_tensor("b_m1", [128, 6], F32, kind="ExternalInput")
    d_b_m2r = nc.dram_tensor("b_m2_rep", [128, 768], F32, kind="ExternalInput")
    d_b_m3 = nc.dram_tensor("b_m3", [128, 3], F32, kind="ExternalInput")
    d_omega3 = nc.dram_tensor("omega3", [128, 3], F32, kind="ExternalInput")
    d_phase3 = nc.dram_tensor("phase3", [128, 3], F32, kind="ExternalInput")
    d_ident = nc.dram_tensor("ident", [128, 128], BF16, kind="ExternalInput")
    d_out = nc.dram_tensor("outT", [3, 128, nseg], F32, kind="ExternalOutput")

    with tile.TileContext(nc) as tc:
        with ExitStack() as ctx:
            ent = ctx.enter_context
            wp = ent(tc.tile_pool(name="wp", bufs=1))
            xin_p = ent(tc.tile_pool(name="xin", bufs=4))
            rep_p = ent(tc.tile_pool(name="rep", bufs=3))
            trig_p = ent(tc.tile_pool(name="trig", bufs=2))
            sc_p = ent(tc.tile_pool(name="sc", bufs=4))
            h1_p = ent(tc.tile_pool(name="h1p", bufs=4))
            h2_p = ent(tc.tile_pool(name="h2p", bufs=4))
            tT_p = ent(tc.tile_pool(name="tTp", bufs=8))
            sel_p = ent(tc.tile_pool(name="selp", bufs=8))
            m2a_p = ent(tc.tile_pool(name="m2ap", bufs=2))
            m2g_p = ent(tc.tile_pool(name="m2gp", bufs=5))
            sm_p = ent(tc.tile_pool(name="smp", bufs=3))
            smT_p = ent(tc.tile_pool(name="smTp", bufs=12))
            out_p = ent(tc.tile_pool(name="outp", bufs=4))
            psA = ent(tc.tile_pool(name="psA", bufs=4, space=bass.MemorySpace.PSUM))
            psE = ent(tc.tile_pool(name="psE", bufs=4, space=bass.MemorySpace.PSUM))

            def wload(dram, shape, dt):
                t = wp.tile(shape, dt, tag=dram.name, name=dram.name + "_sb")
                nc.sync.dma_start(t[:], dram[:])
                return t

            w_in1 = wload(d_w_in1, [16, 384], BF16)
            b_in1 = wload(d_b_in1, [128, 3], F32)
            b_in2 = wload(d_b_in2, [128, 3], F32)
            b_m1 = wload(d_b_m1, [128, 6], F32)
            b_m3 = wload(d_b_m3, [128, 3], F32)
            omega3 = wload(d_omega3, [128, 3], F32)
            phase3 = wload(d_phase3, [128, 3], F32)
            ident = wload(d_ident, [128, 128], BF16)
            w_in2 = wload(d_w_in2, [128, 3, 384], BF16)
            w_m1h = wload(d_w_m1h, [128, 3, 768], BF16)
            w_pe3 = wload(d_w_pe3, [128, 3, 768], BF16)
            b_m2r = wload(d_b_m2r, [128, 768], F32)
            w_m2 = wload(d_w_m2, [128, 6, 768], BF16)
            w_m3 = wload(d_w_m3, [128, 6, 384], BF16)

            pair_ps = [None, None]
            smT_tiles = {}
            pending = []

            def emit_seg(b, m2g):
                selt = sel_p.tile([BIN_E, 2 * BIN_S], BF16, tag="sel", name="sel")
                nc.sync.dma_start(selt[:], d_sel[b])
                half = b % 2
                if half == 0:
                    pair_ps[0] = psE.tile([128, 384], F32, tag="psE", name="psE")
                    pair_ps[1] = psE.tile([128, 384], F32, tag="psE", name="psE")
                pSa, pSb = pair_ps
                # both bins of the pair accumulate into one PSUM pair
                nc.tensor.matmul(pSa[:], selt[:], m2g[:, 0:384],
                                 start=(half == 0), stop=(half == 1))
                nc.tensor.matmul(pSb[:], selt[:], m2g[:, 384:768],
                                 start=(half == 0), stop=(half == 1))
                if half == 0:
                    return
                sm = sm_p.tile([128, 768], BF16, tag="sm", name="sm")
                nc.vector.tensor_copy(sm[:, 0:384], pSa[:])
                nc.vector.tensor_copy(sm[:, 384:768], pSb[:])

                # 128 seg rows complete -> 6 transposes into smeanT
                grp = b // 2
                q = grp % 3
                for kc in range(6):
                    if q == 0:
                        smT_tiles[kc] = smT_p.tile(
                            [128, SEG_BLOCK], BF16, tag="smT", name="smT"
                        )
                    ptr = psA.tile([128, 128], BF16, tag="psA", name="ptr")
                    nc.tensor.transpose(ptr[:], sm[:, bass.ts(kc, 128)], ident[:])
                    nc.vector.tensor_copy(smT_tiles[kc][:, bass.ts(q, 128)], ptr[:])
                if q == 2:
                    sb = grp // 3
                    for j in range(3):
                        ps = psA.tile([128, SEG_BLOCK], F32, tag="psA", name="psA")
                        for kc in range(6):
                            nc.tensor.matmul(
                                ps[:], w_m3[:, kc, bass.ts(j, 128)],
                                smT_tiles[kc][:],
                                start=(kc == 0), stop=(kc == 5),
                            )
                        ot = out_p.tile([128, SEG_BLOCK], F32, tag="out", name="out")
                        nc.vector.tensor_scalar_add(ot[:], ps[:], b_m3[:, j : j + 1])
                        nc.sync.dma_start(d_out[j, :, bass.ts(sb, SEG_BLOCK)], ot[:])


            def trig_reduce(rep_t, c, dst, off):
                """rep3 chunk c -> range-reduced args into dst[:, c, off:]."""
                arg = trig_p.tile([128, TILE_SLOTS], F32, tag="arg", name="arg")
                nc.vector.tensor_scalar(arg[:], rep_t[:, c, :],
                                        omega3[:, c : c + 1],
                                        phase3[:, c : c + 1],
                                        op0=mybir.AluOpType.mult,
                                        op1=mybir.AluOpType.add)
                ki = trig_p.tile([128, TILE_SLOTS], I32, tag="ki", name="ki")
                nc.vector.tensor_scalar_mul(ki[:], arg[:], INV_2PI)
                kf = trig_p.tile([128, TILE_SLOTS], F32, tag="kf", name="kf")
                nc.vector.tensor_copy(kf[:], ki[:])
                nc.vector.cody_waite_cascade(dst[:, c, off : off + TILE_SLOTS],
                                             arg[:], kf[:], CW1, CW2, CW3)

            def pair_front(tis):
                # ---- input DMAs + trig range-reduction (DVE) + ONE SIN for
                # the whole pair (one activation-table swap pair per 2 tiles)
                n = len(tis)
                xTs, reps = [], []
                for ti in tis:
                    xT = xin_p.tile([16, TILE_SLOTS], BF16, tag="xin", name="xin")
                    nc.sync.dma_start(xT[:], d_xT[ti])
                    xTs.append(xT)
                    rep_t = rep_p.tile([128, 3, TILE_SLOTS], F32, tag="rep", name="rep")
                    nc.sync.dma_start(rep_t[:], d_rep3[ti])
                    reps.append(rep_t)
                rr = trig_p.tile([128, 3, n * TILE_SLOTS], F32,
                                 tag=f"rr{n}", name="rr")
                for c in range(3):
                    for k, rep_t in enumerate(reps):
                        trig_reduce(rep_t, c, rr, k * TILE_SLOTS)
                sc = sc_p.tile([128, 3, n * TILE_SLOTS], BF16,
                               tag=f"sc{n}", name="sc")
                nc.scalar.activation(sc[:], rr[:], SIN)
                return xTs, sc

            def tile_body(ti, xT, sc, koff):
                # ---- node MLP (feature-major) ----
                h1 = []
                for j in range(3):
                    ps = psA.tile([128, TILE_SLOTS], F32, tag="psA", name="psA")
                    nc.tensor.matmul(ps[:], w_in1[:, bass.ts(j, 128)], xT[:])
                    t = h1_p.tile([128, TILE_SLOTS], BF16, tag="h1", name="h1")
                    nc.scalar.activation(t[:], ps[:], GELU, bias=b_in1[:, j : j + 1])
                    h1.append(t)
                h2 = []
                for j in range(3):
                    ps = psA.tile([128, TILE_SLOTS], F32, tag="psA", name="psA")
                    for kc in range(3):
                        nc.tensor.matmul(
                            ps[:], w_in2[:, kc, bass.ts(j, 128)], h1[kc][:],
                            start=(kc == 0), stop=(kc == 2),
                        )
                    t = h2_p.tile([128, TILE_SLOTS], BF16, tag="h2", name="h2")
                    nc.scalar.activation(t[:], ps[:], GELU, bias=b_in2[:, j : j + 1])
                    h2.append(t)
                # h3 is fused into w_m1h host-side (no activation in between)
                tT = []
                for f in range(6):
                    ps = psA.tile([128, TILE_SLOTS], F32, tag="psA", name="psA")
                    for kc in range(3):
                        nc.tensor.matmul(
                            ps[:], w_m1h[:, kc, bass.ts(f, 128)], h2[kc][:],
                            start=(kc == 0), stop=False,
                        )
                    for c in range(3):
                        nc.tensor.matmul(
                            ps[:], w_pe3[:, c, bass.ts(f, 128)],
                            sc[:, c, koff : koff + TILE_SLOTS],
                            start=False, stop=(c == 2),
                        )
                    t = tT_p.tile([128, TILE_SLOTS], BF16, tag="tT", name="tT")
                    nc.scalar.activation(t[:], ps[:], GELU, bias=b_m1[:, f : f + 1])
                    tT.append(t)

                # ---- edge stage: 4 bins (m2 part) ----
                for bi in range(BINS_PER_TILE):
                    b = ti * BINS_PER_TILE + bi
                    esl = bass.ts(bi, BIN_E)
                    pA = psE.tile([128, 384], F32, tag="psE", name="psE")
                    pB = psE.tile([128, 384], F32, tag="psE", name="psE")
                    for kc in range(6):
                        nc.tensor.matmul(
                            pA[:], tT[kc][:, esl], w_m2[:, kc, 0:384],
                            start=(kc == 0), stop=(kc == 5),
                        )
                    for kc in range(6):
                        nc.tensor.matmul(
                            pB[:], tT[kc][:, esl], w_m2[:, kc, 384:768],
                            start=(kc == 0), stop=(kc == 5),
                        )
                    m2a = m2a_p.tile([128, 768], F32, tag="m2a", name="m2a")
                    nc.vector.tensor_add(m2a[:, 0:384], pA[:], b_m2r[:, 0:384])
                    nc.vector.tensor_add(m2a[:, 384:768], pB[:], b_m2r[:, 384:768])
                    m2g = m2g_p.tile([128, 768], BF16, tag="m2g", name="m2g")
                    nc.scalar.activation(m2g[:], m2a[:], GELU)
                    pending.append((b, m2g))
                    if len(pending) > 3:
                        emit_seg(*pending.pop(0))

            # pair tiles so the two SINs are adjacent on the scalar queue
            # (one SIN<->GELU activation-table swap pair per TWO tiles)
            for tp in range(0, t_tiles, 2):
                tis = list(range(tp, min(tp + 2, t_tiles)))
                fronts = [pair_front([ti]) for ti in tis]
                for (xTs, sc), ti in zip(fronts, tis):
                    tile_body(ti, xTs[0], sc, 0)
            # drain the pipeline
            while pending:
                emit_seg(*pending.pop(0))
    nc.compile()
    return nc


_NC_CACHE = {}


def _get_nc(nbins):
    if nbins not in _NC_CACHE:
        _NC_CACHE[nbins] = build_nc(nbins)
    return _NC_CACHE[nbins]


def kernel(**inputs):
    x = np.asarray(inputs["x"], np.float32)
    mesh_pos = np.asarray(inputs["mesh_pos"], np.float32)
    grid_pos = np.asarray(inputs["grid_pos"], np.float32)
    edges = np.asarray(inputs["mesh_to_grid_edges"])

    per_core, nbins = pack(edges, x, mesh_pos, grid_pos)
    w = make_weights(inputs)
    nc = _get_nc(nbins)

    common = dict(
        w_in1=w["w_in1"], w_in2=np.ascontiguousarray(w["w_in2"]),
        w_m1h=np.ascontiguousarray(w["w_m1h"]),
        w_pe3=np.ascontiguousarray(w["w_pe3"]),
        w_m2=np.ascontiguousarray(w["w_m2"]),
        w_m3=np.ascontiguousarray(w["w_m3"]),
        b_in1=w["b_in1"], b_in2=w["b_in2"], b_m1=w["b_m1"],
        b_m2_rep=w["b_m2_rep"], b_m3=w["b_m3"],
        omega3=w["omega3"], phase3=w["phase3"], ident=w["ident"],
    )
    in_maps = [dict(common, xT_t=pc["xT_t"], rep3_t=pc["rep3_t"],
                    sel_t=pc["sel"]) for pc in per_core]

    res = bass_utils.run_bass_kernel_spmd(nc, in_maps, core_ids=list(range(N_CORES)))
    outs_rows = [r["outT"].transpose(2, 0, 1).reshape(per_core[0]["nseg"], HID)
                 for r in res.results]
    return assemble(per_core, outs_rows, np.asarray(inputs["b_m3"], np.float32))

